# revision 8
# baseline (speedup 1.0000x reference)
"""Masked max-pool over span axis (MaxSpanRepr) on 8 Trainium2 cores.

Computation: out[b, l, d] = max_s( mask[b, s] ? spans[b, l, s, d] : -1e10 )
  spans          [2048, 13, 4, 1024] f32
  attention_mask [2048, 4] int32
  out            [2048, 13, 1024] f32

Strategy: data-parallel over batch, 256 examples per core. Per core the
spans shard is a [13312 x 1024] table of 4KB chunks (chunk index
r*4 + s for row r=(b,l)); row r needs the max over its k=popcount(mask)
valid chunks, so the memory floor is reading exactly the valid chunks
(~half the dense bytes) plus the output write.

Shared NEFF structure (one NEFF, 8 SPMD cores): a list of 128-row
tiles, each with a fixed chunk count k_t, sized so that every core can
pack its rows into tiles with k_t >= k(row): #tiles(k_t >= j) =
ceil(max_core #rows(k >= j) / 128). Each core assigns its own rows to
tiles (sorted by k descending) and ships per-core index tables; rows
with k < k_t re-read their first chunk in the pad groups (max(x,x)=x).
The slowest core is the one with the most valid chunks, and its
padding is minimal, so the makespan stays at the memory floor.

Device pipeline per window of tiles (sum k_t <= G_MAX groups):
  - ONE dma_gather (SWDGE custom instruction, int16 index stream, 4KB
    elements): gathers 128*G_w chunks; stream position j lands in
    partition j%128, group j//128, so tile t's rows occupy groups
    [o_t, o_t+k_t) partition-aligned. One instruction replaces ~3*G_w
    indirect DMAs (descriptor-gen on the gpsimd engine was the
    previous bottleneck at ~1.4us each).
  - per tile: (k_t-1) vector (add,max) scalar_tensor_tensor ops reduce
    the groups (k_t=1 tiles skip compute entirely).
  - an indirect scatter-DMA stores each tile's 128 rows back to their
    original positions (4KB per partition).

k=0 rows output exactly -1e10: they are written by indirect scatters
from a const -1e10 tile (full const tiles plus a few overflow columns
for k=0 rows that land in gather tiles, whose main-scatter slots are
OOB-skipped). The NEFF is recompiled if a different mask changes the
structure (cached by structure key).
"""

import math

import numpy as np

import concourse.bass as bass
import concourse.mybir as mybir
from concourse.bass_utils import run_bass_kernel_spmd
from concourse.library_overlay import lower_extended_insts
from concourse.tile import TileContext

B, L, S, D = 2048, 13, 4, 1024
N_CORES = 8
B_SH = B // N_CORES              # 256 examples per core
ROWS = B_SH * L                  # 3328 (b,l) rows per core
N_CHUNKS = ROWS * S              # 13312 4KB chunks per core
N_TILES = ROWS // 128            # 26 tiles of 128 rows
G_MAX = 6                        # gather-window size in 4KB groups
NEG_FILL = np.float32(-1e10)
OOB_IDX = np.int32(10 ** 7)      # scatter skip marker

_NC_CACHE = {}


# The walrus build in this container supports a single sync-wait slot per
# instruction ("Too many sync wait commands" in setupSyncWait otherwise),
# while Tile freely attaches one wait per semaphore lane. Post-pass: for any
# instruction carrying N>1 waits, hoist N-1 of them onto NoOp instructions
# inserted just before it on the same engine (engines execute in order, so
# all waits still complete before the instruction runs).
def _split_multi_wait_instructions(nc):
    ctr = 0
    for fn in nc.m.functions:
        for blk in fn.blocks:
            insts = blk.instructions
            out = []
            changed = False
            for inst in insts:
                si = inst.sync_info
                waits = list(si.on_wait) if si is not None else []
                if len(waits) > 1:
                    changed = True
                    for w in waits[:-1]:
                        ctr += 1
                        nop = mybir.InstNoOp(
                            name=f"I-waitsplit-{ctr}", ins=[], outs=[])
                        nop.engine = inst.engine
                        nsi = mybir.SyncInfo(on_update=[], on_wait=[w])
                        nop.sync_info = nsi
                        out.append(nop)
                    si.on_wait = [waits[-1]]
                out.append(inst)
            if changed:
                blk.instructions = out


def _plan(k_all):
    """Shared NEFF structure from per-core row chunk-counts.
    k_all: [N_CORES, ROWS] ints in 0..4 (unsorted).
    Returns (k_structs, n_const_tiles, n_bnd, windows):
      k_structs: descending per-gather-tile group counts
      windows: list of (tiles=[(tile_idx, o_t, k_t), ...], G_w)."""
    n_ge = np.array([[(k_all[c] >= j).sum() for j in range(1, S + 1)]
                     for c in range(N_CORES)])          # [C, 4]
    tiles_ge = [math.ceil(int(n_ge[:, j - 1].max()) / 128)
                for j in range(1, S + 1)] + [0]
    k_structs = []
    for j in range(S, 0, -1):                           # descending k
        k_structs += [j] * (tiles_ge[j - 1] - tiles_ge[j])
    n_gather = len(k_structs)                           # = tiles_ge[0]
    assert n_gather <= N_TILES
    n_const_tiles = N_TILES - n_gather
    # k=0 rows that must live inside gather tiles (overflow columns)
    leftover0 = 128 * n_gather - n_ge[:, 0]
    n_bnd = max(math.ceil(int(leftover0.max()) / 128), 0)

    windows = []
    cur, g = [], 0
    for t, kt in enumerate(k_structs):
        if g + kt > G_MAX and cur:
            windows.append((cur, g))
            cur, g = [], 0
        cur.append((t, g, kt))
        g += kt
    if cur:
        windows.append((cur, g))
    return k_structs, n_const_tiles, n_bnd, windows


def _build_nc(k_structs, n_const_tiles, n_bnd, windows):
    key = (tuple(k_structs), n_const_tiles, n_bnd)
    if key in _NC_CACHE:
        return _NC_CACHE[key]
    from concourse import library_config

    total_cols16 = sum(8 * gw for _, gw in windows)
    n_gather = len(k_structs)
    n_scat_cols = n_gather + n_const_tiles + n_bnd

    nc = bass.Bass()
    f32, i16 = mybir.dt.float32, mybir.dt.int16
    i32 = mybir.dt.int32
    spans = nc.dram_tensor("spans", [N_CHUNKS, D], f32, kind="ExternalInput")
    gidx = nc.dram_tensor("gidx", [128, total_cols16], i16,
                          kind="ExternalInput")
    rowid = nc.dram_tensor("rowid", [128, n_scat_cols], i32,
                           kind="ExternalInput")
    out = nc.dram_tensor("out", [ROWS, D], f32, kind="ExternalOutput")

    with TileContext(nc) as tc:
        with (
            tc.tile_pool(name="constp", bufs=1) as const_pool,
            tc.tile_pool(name="dstp", bufs=3) as dst_pool,
            tc.tile_pool(name="outp", bufs=6) as out_pool,
        ):
            gidx_t = const_pool.tile([128, total_cols16], i16)
            nc.sync.dma_start(out=gidx_t[:], in_=gidx[:])
            rowid_t = const_pool.tile([128, n_scat_cols], i32)
            nc.sync.dma_start(out=rowid_t[:], in_=rowid[:])
            neg_t = const_pool.tile([128, D], f32)
            nc.vector.memset(neg_t[:], float(NEG_FILL))

            nc.gpsimd.load_library(library_config.mlp)
            bounds_rows = nc.gpsimd.to_reg(ROWS - 1)

            def scatter(src_ap, col):
                nc.gpsimd.indirect_dma_start(
                    out=out[:],
                    out_offset=bass.IndirectOffsetOnAxis(
                        ap=rowid_t[:, col:col + 1], axis=0),
                    in_=src_ap,
                    in_offset=None,
                    bounds_check=bounds_rows,
                    oob_is_err=False,
                )

            # k=0 rows first: const -1e10 tiles + overflow columns. No
            # gather dependency, so these fill the pipeline ramp.
            for c in range(n_const_tiles + n_bnd):
                scatter(neg_t[:], n_gather + c)

            def compute_scatter(tiles, dst):
                for (t, o_t, k_t) in tiles:
                    if k_t == 1:
                        src = dst[:, o_t, :]
                    else:
                        tout = out_pool.tile([128, D], f32, tag="tout")
                        nc.vector.scalar_tensor_tensor(
                            out=tout[:], in0=dst[:, o_t, :],
                            scalar=0.0, in1=dst[:, o_t + 1, :],
                            op0=mybir.AluOpType.add,
                            op1=mybir.AluOpType.max,
                        )
                        for j in range(2, k_t):
                            nc.vector.scalar_tensor_tensor(
                                out=tout[:], in0=dst[:, o_t + j, :],
                                scalar=0.0, in1=tout[:],
                                op0=mybir.AluOpType.add,
                                op1=mybir.AluOpType.max,
                            )
                        src = tout[:]
                    scatter(src, t)

            # one-window issue skew: the next window's gather goes out
            # before the previous window's compute+stores
            off16 = 0
            prev = None
            for (tiles, gw) in windows:
                dst = dst_pool.tile([128, G_MAX, D], f32, tag="dst")
                nc.gpsimd.dma_gather(
                    dst[:, 0:gw, :], spans[:],
                    gidx_t[:, off16:off16 + 8 * gw],
                    128 * gw, 128 * gw, D)
                off16 += 8 * gw
                if prev is not None:
                    compute_scatter(*prev)
                prev = (tiles, dst)
            compute_scatter(*prev)

    lower_extended_insts(nc)
    _split_multi_wait_instructions(nc)
    _NC_CACHE[key] = nc
    return nc


def _core_tables(valid_core, k_structs, n_const_tiles, n_bnd, windows):
    """Per-core gather/scatter tables for the shared structure.
    valid_core: [ROWS, S] bool. Returns (gidx [128, cols16] i16,
    rowid [128, n_gather + n_const_tiles + n_bnd] i32)."""
    n_gather = len(k_structs)
    k_r = valid_core.sum(1)
    order = np.argsort(-k_r, kind="stable")         # descending k
    k_sorted = k_r[order]
    # chunk ids per row, padded with the row's first chunk
    chunks = np.nonzero(valid_core[order])          # sorted-row major
    first = np.zeros(ROWS, np.int64)
    vs_list = [[] for _ in range(ROWS)]
    for rr, s in zip(*chunks):
        vs_list[rr].append(order[rr] * S + s)

    rowid = np.full((128, n_gather + n_const_tiles + n_bnd), OOB_IDX,
                    np.int32)
    # gather tiles: sorted rows [128t, 128t+128); OOB for k=0 rows
    for t in range(n_gather):
        rows = order[t * 128:(t + 1) * 128]
        live = k_sorted[t * 128:(t + 1) * 128] > 0
        rowid[live, t] = rows[live]
    # const coverage: all k=0 rows (those beyond the gather range, plus
    # the gather-range stragglers via overflow columns)
    zeros = order[np.nonzero(k_sorted == 0)[0]]
    for i in range(0, len(zeros), 128):
        blk = zeros[i:i + 128]
        col = n_gather + i // 128
        rowid[:len(blk), col] = blk

    stream = np.empty(sum(128 * gw for _, gw in windows), np.int16)
    pos = 0
    for (tiles, gw) in windows:
        for (t, o_t, k_t) in tiles:
            base = t * 128
            for j in range(k_t):
                for p in range(128):
                    vs = vs_list[base + p]
                    if not vs:
                        stream[pos] = 0             # pad; scatter skips row
                    elif j < len(vs):
                        stream[pos] = vs[j]
                    else:
                        stream[pos] = vs[0]         # dup pad
                    pos += 1
    cols16 = len(stream) // 16
    gidx16 = np.zeros((16, cols16), np.int16)
    ppos = np.arange(len(stream))
    gidx16[ppos % 16, ppos // 16] = stream
    gidx = np.tile(gidx16, (8, 1))                  # 8 Q7 cores
    return gidx, rowid


def _make_all(spans, attention_mask):
    spans = np.ascontiguousarray(np.asarray(spans, dtype=np.float32))
    mask = np.asarray(attention_mask)
    assert spans.shape == (B, L, S, D), spans.shape
    assert mask.shape == (B, S), mask.shape

    valid = mask != 0                                    # [B, S]
    spans_flat = spans.reshape(B * L, S * D)

    valid_cores = []
    k_all = np.empty((N_CORES, ROWS), np.int64)
    for i in range(N_CORES):
        vc = np.repeat(valid[i * B_SH:(i + 1) * B_SH], L, axis=0)
        valid_cores.append(vc)
        k_all[i] = vc.sum(1)
    plan = _plan(k_all)

    in_maps = []
    for i in range(N_CORES):
        gidx, rowid = _core_tables(valid_cores[i], *plan)
        sl = slice(i * ROWS, (i + 1) * ROWS)
        in_maps.append({
            "spans": spans_flat[sl].reshape(ROWS * S, D),
            "gidx": gidx,
            "rowid": rowid,
        })
    return plan, in_maps


def run(spans, attention_mask, **spmd_kwargs):
    """Run the device kernel; returns (full_output, BassKernelResults)."""
    plan, in_maps = _make_all(spans, attention_mask)
    nc = _build_nc(*plan)
    res = run_bass_kernel_spmd(nc, in_maps, core_ids=list(range(N_CORES)),
                               **spmd_kwargs)
    outs = [r["out"] for r in res.results]
    full = np.concatenate(outs, axis=0).reshape(B, L, D)
    return full, res


def kernel(spans, attention_mask):
    full, _ = run(spans, attention_mask)
    return full


# revision 11
# speedup vs baseline: 1.5813x; 1.5813x over previous
"""Masked max-pool over span axis (MaxSpanRepr) on 8 Trainium2 cores.

Computation: out[b, l, d] = max_s( mask[b, s] ? spans[b, l, s, d] : -1e10 )
  spans          [2048, 13, 4, 1024] f32
  attention_mask [2048, 4] int32
  out            [2048, 13, 1024] f32

Strategy: data-parallel over batch, 256 examples per core. Per core the
spans shard is a [13312 x 1024] table of 4KB chunks (chunk index
r*4 + s for row r=(b,l)); row r needs the max over its k=popcount(mask)
valid chunks, so the memory floor is reading exactly the valid chunks
(~half the dense bytes) plus the output write.

Unit of work: a PAIR of consecutive rows (2p, 2p+1) - 12 of 13 pairs
share the same example b, hence the same mask and k. Pairs are sorted
by K = max(k0, k1) so each 128-pair tile has a uniform group count
K_t; the shared NEFF structure is sized from per-K pair counts maxed
across cores (each core packs its own pairs, so the heaviest core -
the makespan - pads least).

Device pipeline per tile window (sum 2*K_t <= 8 groups):
  - dma_gather instructions (SWDGE custom op, int16 index stream, 4KB
    elements, <=512 indices each): stream position j lands in
    partition j%128, group j//128; tile t's pair rows occupy groups
    [o_t, o_t+K_t) and [o_t+K_t, o_t+2K_t). Replaces per-slot indirect
    DMAs whose descriptor generation dominated the gpsimd engine.
  - per tile: 2*(K_t-1) vector (add,max) scalar_tensor_tensor ops
    reduce each row's groups (K_t=1 tiles use one scalar-engine copy).
  - ONE indirect scatter-DMA per tile stores [128, 2, D] to rows
    (2p, 2p+1) - consecutive in the original order, so one 8KB extent
    per partition. Tile serializes dynamic DRAM writes into a
    semaphore chain (gen waits the previous scatter's transfer), so
    halving the scatter count keeps the chain under the gather span.

k=0 rows output exactly -1e10: all-k0 pairs are written by scatters
from a const -1e10 tile (OOB-skipped in the main scatters); mixed
pairs (a k=0 row next to a live row, only at example boundaries) are
patched by a final 4KB const scatter overwriting just those rows.
Rows with k < K_t re-read their first chunk in pad groups
(max(x,x)=x). The NEFF is recompiled if a different mask changes the
structure (cached by structure key).
"""

import math

import numpy as np

import concourse.bass as bass
import concourse.mybir as mybir
from concourse.bass_utils import run_bass_kernel_spmd
from concourse.library_overlay import lower_extended_insts
from concourse.tile import TileContext

B, L, S, D = 2048, 13, 4, 1024
N_CORES = 8
B_SH = B // N_CORES              # 256 examples per core
ROWS = B_SH * L                  # 3328 (b,l) rows per core
N_CHUNKS = ROWS * S              # 13312 4KB chunks per core
N_PAIRS = ROWS // 2              # 1664 row-pairs per core
N_PTILES = N_PAIRS // 128        # 13 pair-tiles
G_MAX = 8                        # window size in 4KB groups
G_SUB = 4                        # max groups per dma_gather instruction
NEG_FILL = np.float32(-1e10)
OOB_IDX = np.int32(10 ** 7)      # scatter skip marker

_NC_CACHE = {}


# The walrus build in this container supports a single sync-wait slot per
# instruction ("Too many sync wait commands" in setupSyncWait otherwise),
# while Tile freely attaches one wait per semaphore lane. Post-pass: for any
# instruction carrying N>1 waits, hoist N-1 of them onto NoOp instructions
# inserted just before it on the same engine (engines execute in order, so
# all waits still complete before the instruction runs).
def _split_multi_wait_instructions(nc):
    ctr = 0
    for fn in nc.m.functions:
        for blk in fn.blocks:
            insts = blk.instructions
            out = []
            changed = False
            for inst in insts:
                si = inst.sync_info
                waits = list(si.on_wait) if si is not None else []
                if len(waits) > 1:
                    changed = True
                    for w in waits[:-1]:
                        ctr += 1
                        nop = mybir.InstNoOp(
                            name=f"I-waitsplit-{ctr}", ins=[], outs=[])
                        nop.engine = inst.engine
                        nsi = mybir.SyncInfo(on_update=[], on_wait=[w])
                        nop.sync_info = nsi
                        out.append(nop)
                    si.on_wait = [waits[-1]]
                out.append(inst)
            if changed:
                blk.instructions = out


def windows_pack(K_structs):
    windows = []
    cur, g = [], 0
    for t, kt in enumerate(K_structs):
        gt = 2 * kt
        if g + gt > G_MAX and cur:
            windows.append((cur, g))
            cur, g = [], 0
        cur.append((t, g, kt))
        g += gt
    if cur:
        windows.append((cur, g))
    return windows


def _build_nc(K_structs, n_const_cols, n_half_cols, windows):
    key = (tuple(K_structs), n_const_cols, n_half_cols)
    if key in _NC_CACHE:
        return _NC_CACHE[key]
    from concourse import library_config

    total_cols16 = sum(8 * gw for _, gw in windows)
    n_gather = len(K_structs)

    nc = bass.Bass()
    f32, i16 = mybir.dt.float32, mybir.dt.int16
    i32 = mybir.dt.int32
    spans = nc.dram_tensor("spans", [N_CHUNKS, D], f32, kind="ExternalInput")
    gidx = nc.dram_tensor("gidx", [128, total_cols16], i16,
                          kind="ExternalInput")
    rowid = nc.dram_tensor("rowid", [128, 2 * n_gather], i32,
                           kind="ExternalInput")
    constid = nc.dram_tensor("constid", [128, 2 * n_const_cols], i32,
                             kind="ExternalInput")
    halfid = nc.dram_tensor("halfid", [128, n_half_cols], i32,
                            kind="ExternalInput")
    out = nc.dram_tensor("out", [ROWS, D], f32, kind="ExternalOutput")

    with TileContext(nc) as tc:
        with (
            tc.tile_pool(name="constp", bufs=1) as const_pool,
            tc.tile_pool(name="dstp", bufs=3) as dst_pool,
            tc.tile_pool(name="outp", bufs=6) as out_pool,
        ):
            gidx_t = const_pool.tile([128, total_cols16], i16)
            nc.sync.dma_start(out=gidx_t[:], in_=gidx[:])
            rowid_t = const_pool.tile([128, 2 * n_gather], i32)
            nc.sync.dma_start(out=rowid_t[:], in_=rowid[:])
            constid_t = const_pool.tile([128, 2 * n_const_cols], i32)
            nc.sync.dma_start(out=constid_t[:], in_=constid[:])
            halfid_t = const_pool.tile([128, n_half_cols], i32)
            nc.sync.dma_start(out=halfid_t[:], in_=halfid[:])
            neg_t = const_pool.tile([128, 2, D], f32)
            nc.vector.memset(neg_t[:], float(NEG_FILL))

            nc.gpsimd.load_library(library_config.mlp)
            bounds_rows = nc.gpsimd.to_reg(ROWS - 1)
            scatter_insts = []

            def scatter(src_ap, idx_ap, prune=True):
                inst = nc.gpsimd.indirect_dma_start(
                    out=out[:],
                    out_offset=bass.IndirectOffsetOnAxis(ap=idx_ap, axis=0),
                    in_=src_ap,
                    in_offset=None,
                    bounds_check=bounds_rows,
                    oob_is_err=False,
                )
                if prune:
                    scatter_insts.append(inst.ins
                                         if hasattr(inst, "ins") else inst)

            # all-k0 pairs first (no gather dependency, fills the ramp)
            for c in range(n_const_cols):
                scatter(neg_t[:, 0, :], constid_t[:, 2 * c:2 * c + 1])
                scatter(neg_t[:, 1, :],
                        constid_t[:, 2 * c + 1:2 * c + 2])

            def compute_scatter(tiles, dst):
                for (t, o_t, k_t) in tiles:
                    stout = out_pool.tile([128, 2, D], f32, tag="stout")
                    if k_t == 1:
                        nc.scalar.copy(out=stout[:],
                                       in_=dst[:, o_t:o_t + 2, :])
                    else:
                        for half in (0, 1):
                            base = o_t + half * k_t
                            nc.vector.scalar_tensor_tensor(
                                out=stout[:, half, :],
                                in0=dst[:, base, :],
                                scalar=0.0, in1=dst[:, base + 1, :],
                                op0=mybir.AluOpType.add,
                                op1=mybir.AluOpType.max,
                            )
                            for j in range(2, k_t):
                                nc.vector.scalar_tensor_tensor(
                                    out=stout[:, half, :],
                                    in0=dst[:, base + j, :],
                                    scalar=0.0, in1=stout[:, half, :],
                                    op0=mybir.AluOpType.add,
                                    op1=mybir.AluOpType.max,
                                )
                    scatter(stout[:, 0, :], rowid_t[:, 2 * t:2 * t + 1])
                    scatter(stout[:, 1, :],
                            rowid_t[:, 2 * t + 1:2 * t + 2])

            # one-window issue skew: the next window's gathers go out
            # before the previous window's compute+stores
            off16 = 0
            prev = None
            for (tiles, gw) in windows:
                dst = dst_pool.tile([128, G_MAX, D], f32, tag="dst")
                a = 0
                while a < gw:
                    g = min(G_SUB, gw - a)
                    nc.gpsimd.dma_gather(
                        dst[:, a:a + g, :], spans[:],
                        gidx_t[:, off16:off16 + 8 * g],
                        128 * g, 128 * g, D)
                    off16 += 8 * g
                    a += g
                if prev is not None:
                    compute_scatter(*prev)
                prev = (tiles, dst)
            compute_scatter(*prev)

            # patch k0 rows living inside live pairs (example-boundary
            # pairs): must come after the main scatters (WAW)
            for c in range(n_half_cols):
                scatter(neg_t[:, 0, :], halfid_t[:, c:c + 1], prune=False)

        names = {i.name for i in scatter_insts}
        for inst in scatter_insts:
            for dep in list(inst.sync_dependency_names()):
                if dep in names:
                    inst.try_remove_dependency(dep)

    lower_extended_insts(nc)
    _split_multi_wait_instructions(nc)
    _NC_CACHE[key] = nc
    return nc


def _core_tables(valid_core, K_structs, n_const_cols, n_half_cols, windows):
    """Per-core tables. valid_core: [ROWS, S] bool."""
    n_gather = len(K_structs)
    k_r = valid_core.sum(1)
    K_p = np.maximum(k_r[0::2], k_r[1::2])          # [N_PAIRS]
    order_p = np.argsort(-K_p, kind="stable")       # descending K
    K_sorted = K_p[order_p]

    vs_list = [None] * ROWS
    rr, ss = np.nonzero(valid_core)
    for r, s in zip(rr, ss):
        if vs_list[r] is None:
            vs_list[r] = []
        vs_list[r].append(r * S + s)

    rowid = np.full((128, 2 * n_gather), OOB_IDX, np.int32)
    for t in range(n_gather):
        prs = order_p[t * 128:(t + 1) * 128]
        live = K_sorted[t * 128:(t + 1) * 128] > 0
        rowid[live, 2 * t] = (2 * prs[live]).astype(np.int32)
        rowid[live, 2 * t + 1] = (2 * prs[live] + 1).astype(np.int32)

    constid = np.full((128, 2 * n_const_cols), OOB_IDX, np.int32)
    zeros = order_p[np.nonzero(K_sorted == 0)[0]]
    assert len(zeros) <= 128 * n_const_cols, (len(zeros), n_const_cols)
    for i in range(0, len(zeros), 128):
        blk = zeros[i:i + 128]
        constid[:len(blk), 2 * (i // 128)] = 2 * blk
        constid[:len(blk), 2 * (i // 128) + 1] = 2 * blk + 1

    halfid = np.full((128, n_half_cols), OOB_IDX, np.int32)
    half_rows = []
    live_pairs = order_p[np.nonzero(K_sorted > 0)[0]]
    for pr in live_pairs:
        for r in (2 * pr, 2 * pr + 1):
            if k_r[r] == 0:
                half_rows.append(r)
    assert len(half_rows) <= 128 * n_half_cols, (len(half_rows), n_half_cols)
    for i in range(0, len(half_rows), 128):
        blk = half_rows[i:i + 128]
        halfid[:len(blk), i // 128] = blk

    stream = np.empty(sum(128 * gw for _, gw in windows), np.int16)
    pos = 0
    for (tiles, gw) in windows:
        for (t, o_t, k_t) in tiles:
            prs = order_p[t * 128:(t + 1) * 128]
            for j in range(2 * k_t):
                half, jj = (1, j - k_t) if j >= k_t else (0, j)
                for p in range(128):
                    r = 2 * prs[p] + half
                    vs = vs_list[r]
                    if not vs:
                        stream[pos] = 0             # pad; fixed by const
                    elif jj < len(vs):
                        stream[pos] = vs[jj]
                    else:
                        stream[pos] = vs[0]         # dup pad
                    pos += 1
    assert pos == len(stream)
    cols16 = len(stream) // 16
    gidx16 = np.zeros((16, cols16), np.int16)
    ppos = np.arange(len(stream))
    gidx16[ppos % 16, ppos // 16] = stream
    gidx = np.tile(gidx16, (8, 1))                  # 8 Q7 cores
    return gidx, rowid, constid, halfid


def _make_all(spans, attention_mask):
    spans = np.ascontiguousarray(np.asarray(spans, dtype=np.float32))
    mask = np.asarray(attention_mask)
    assert spans.shape == (B, L, S, D), spans.shape
    assert mask.shape == (B, S), mask.shape

    valid = mask != 0                                    # [B, S]
    spans_flat = spans.reshape(B * L, S * D)

    valid_cores = []
    K_all = np.empty((N_CORES, N_PAIRS), np.int64)
    n_half = np.zeros(N_CORES, np.int64)
    for i in range(N_CORES):
        vc = np.repeat(valid[i * B_SH:(i + 1) * B_SH], L, axis=0)
        valid_cores.append(vc)
        k_r = vc.sum(1)
        K_all[i] = np.maximum(k_r[0::2], k_r[1::2])
        n_half[i] = int(np.sum((K_all[i] > 0)
                               & (np.minimum(k_r[0::2], k_r[1::2]) == 0)))

    n_ge = np.array([[(K_all[c] >= j).sum() for j in range(1, S + 1)]
                     for c in range(N_CORES)])
    tiles_ge = [math.ceil(int(n_ge[:, j - 1].max()) / 128)
                for j in range(1, S + 1)] + [0]
    K_structs = []
    for j in range(S, 0, -1):
        K_structs += [j] * (tiles_ge[j - 1] - tiles_ge[j])
    n0 = N_PAIRS - n_ge[:, 0]
    n_const_cols = max(math.ceil(int(n0.max()) / 128), 1)
    n_half_cols = max(math.ceil(int(n_half.max()) / 128), 1)
    windows = windows_pack(K_structs)
    plan = (K_structs, n_const_cols, n_half_cols, windows)

    in_maps = []
    for i in range(N_CORES):
        gidx, rowid, constid, halfid = _core_tables(valid_cores[i], *plan)
        sl = slice(i * ROWS, (i + 1) * ROWS)
        in_maps.append({
            "spans": spans_flat[sl].reshape(ROWS * S, D),
            "gidx": gidx,
            "rowid": rowid,
            "constid": constid,
            "halfid": halfid,
        })
    return plan, in_maps


def run(spans, attention_mask, **spmd_kwargs):
    """Run the device kernel; returns (full_output, BassKernelResults)."""
    plan, in_maps = _make_all(spans, attention_mask)
    nc = _build_nc(*plan)
    res = run_bass_kernel_spmd(nc, in_maps, core_ids=list(range(N_CORES)),
                               **spmd_kwargs)
    outs = [r["out"] for r in res.results]
    full = np.concatenate(outs, axis=0).reshape(B, L, D)
    return full, res


def kernel(spans, attention_mask):
    full, _ = run(spans, attention_mask)
    return full
